# revision 1
# baseline (speedup 1.0000x reference)
"""MoE (top-2 of 8 experts) Trainium2 kernel.

Strategy: expert-parallel across the 8 NeuronCores. The (cheap) router runs
on host CPU; the host gathers each expert's routed tokens (already
transposed to [D, C] so the device needs no transposes), each core runs the
dense expert MLP  y = (silu(x @ w1_e) @ w2_e) * combine_weight  for its
expert's tokens only (~T*K/E tokens instead of all T — a 4x compute saving
over the dense formulation), and the host scatter-adds the per-expert
contributions back into the full [B,S,D] output.

Self-contained: only environment packages (numpy/jax/concourse) are used.
"""

import os
import sys

import numpy as np

sys.path.insert(0, "/opt/trn_rl_repo")

B, S, D_MODEL, D_FF, N_EXPERTS, TOP_K = 2, 2048, 1024, 2048, 8, 2
T = B * S
N_CORES = 8

# compute dtype for the expert MLP matmuls: "bf16", "f32", or "f32r"
COMPUTE_DTYPE = os.environ.get("BASS_MOE_DTYPE", "bf16")

_PROGRAM_CACHE: dict = {}
LAST_BUILD = {}


def _round_up(v: int, m: int) -> int:
    return ((v + m - 1) // m) * m


def _blocks(C: int):
    """Token blocks of <=512 (PSUM-bank limit on matmul free dim)."""
    out = []
    b0 = 0
    while b0 < C:
        bs = min(512, C - b0)
        out.append((b0, bs))
        b0 += bs
    return out


def _build_program(C: int, cdtype: str):
    """Build + compile the per-core expert-MLP program for capacity C."""
    import concourse.tile as tile
    from concourse import bacc, mybir

    if cdtype == "bf16":
        mdt = mybir.dt.bfloat16
    elif cdtype == "f32":
        mdt = mybir.dt.float32
    elif cdtype == "f32r":
        mdt = mybir.dt.float32r
    else:
        raise ValueError(cdtype)

    f32 = mybir.dt.float32
    KD = D_MODEL // 128   # 8  k-chunks for matmul 1
    KF = D_FF // 128      # 16 k-chunks for matmul 2

    nc = bacc.Bacc("TRN2", target_bir_lowering=False, debug=False,
                   num_devices=N_CORES)
    xt_d = nc.dram_tensor("xt", [D_MODEL, C], mdt, kind="ExternalInput").ap()
    w1_d = nc.dram_tensor("w1", [D_MODEL, D_FF], mdt, kind="ExternalInput").ap()
    w2_d = nc.dram_tensor("w2", [D_FF, D_MODEL], mdt, kind="ExternalInput").ap()
    cw_d = nc.dram_tensor("cw", [C], f32, kind="ExternalInput").ap()
    y_d = nc.dram_tensor("y", [C, D_MODEL], f32, kind="ExternalOutput").ap()

    silu = mybir.ActivationFunctionType.Silu

    with tile.TileContext(nc) as tc:
        with (
            tc.tile_pool(name="wpool", bufs=1) as wpool,
            tc.tile_pool(name="xpool", bufs=2) as xpool,
            tc.tile_pool(name="hpool", bufs=2) as hpool,
            tc.tile_pool(name="cwpool", bufs=3) as cwpool,
            tc.tile_pool(name="ypool", bufs=3) as ypool,
            tc.tile_pool(name="pspool", bufs=1, space="PSUM") as pspool,
        ):
            # resident weights
            w1_sb = []
            for kk in range(KD):
                t = wpool.tile([128, D_FF], mdt, tag=f"w1_{kk}")
                nc.sync.dma_start(t[:], w1_d[kk * 128:(kk + 1) * 128, :])
                w1_sb.append(t)
            w2_sb = []
            for kk in range(KF):
                t = wpool.tile([128, D_MODEL], mdt, tag=f"w2_{kk}")
                nc.sync.dma_start(t[:], w2_d[kk * 128:(kk + 1) * 128, :])
                w2_sb.append(t)

            for (b0, bs) in _blocks(C):
                # stream this block's tokens in (already transposed on host)
                xt_sb = []
                for kk in range(KD):
                    t = xpool.tile([128, bs], mdt, tag=f"x{kk}")
                    nc.sync.dma_start(
                        t[:], xt_d[kk * 128:(kk + 1) * 128, b0:b0 + bs])
                    xt_sb.append(t)

                # h[ff, tok] = silu(w1.T-slice @ x)  — stored transposed so it
                # can be the stationary operand of matmul 2
                h_sb = []
                for fm in range(KF):
                    ps = pspool.tile([128, bs], f32, tag="ph", bufs=3)
                    for kk in range(KD):
                        nc.tensor.matmul(
                            ps[:],
                            lhsT=w1_sb[kk][:, fm * 128:(fm + 1) * 128],
                            rhs=xt_sb[kk][:],
                            start=(kk == 0), stop=(kk == KD - 1))
                    h = hpool.tile([128, bs], mdt, tag=f"h{fm}")
                    nc.scalar.activation(h[:], ps[:], silu)
                    h_sb.append(h)

                # y[tok, d] = (h.T @ w2) * cw[tok]
                for tm in range(bs // 128):
                    t0 = b0 + tm * 128
                    cwt = cwpool.tile([128, 1], f32, tag="cw")
                    nc.sync.dma_start(
                        cwt[:], cw_d[t0:t0 + 128].rearrange("(p o) -> p o", o=1))
                    ys = ypool.tile([128, D_MODEL], f32, tag="y")
                    for dn in range(D_MODEL // 512):
                        ps = pspool.tile([128, 512], f32, tag="py", bufs=3)
                        for fk in range(KF):
                            nc.tensor.matmul(
                                ps[:],
                                lhsT=h_sb[fk][:, tm * 128:(tm + 1) * 128],
                                rhs=w2_sb[fk][:, dn * 512:(dn + 1) * 512],
                                start=(fk == 0), stop=(fk == KF - 1))
                        # out = psum * combine_weight (per-partition scalar)
                        nc.scalar.mul(ys[:, dn * 512:(dn + 1) * 512], ps[:],
                                      cwt[:])
                    nc.sync.dma_start(y_d[t0:t0 + 128, :], ys[:])

    nc.compile()
    return nc


def _route(x: np.ndarray, gate_w: np.ndarray):
    """Router on host CPU with the reference's exact jax ops/dtypes."""
    try:
        import jax
        import jax.numpy as jnp
        with jax.default_device(jax.devices("cpu")[0]):
            logits = jnp.einsum('bsd,de->bse', jnp.asarray(x),
                                jnp.asarray(gate_w))
            top_logits, top_idx = jax.lax.top_k(logits, TOP_K)
            top_w = jax.nn.softmax(top_logits, axis=-1)
            ti = np.asarray(top_idx).reshape(T, TOP_K)
            tw = np.asarray(top_w).reshape(T, TOP_K).astype(np.float32)
    except Exception:
        # numpy fallback (same selection semantics as jax.lax.top_k)
        logits = (x.reshape(T, D_MODEL) @ gate_w).astype(np.float32)
        i0 = np.argmax(logits, axis=1)
        masked = logits.copy()
        masked[np.arange(T), i0] = -np.inf
        i1 = np.argmax(masked, axis=1)
        v0 = logits[np.arange(T), i0]
        v1 = logits[np.arange(T), i1]
        e1 = np.exp(v1 - v0)
        w0 = 1.0 / (1.0 + e1)
        ti = np.stack([i0, i1], 1)
        tw = np.stack([w0, 1.0 - w0], 1).astype(np.float32)
    return ti, tw


def kernel(x: np.ndarray, gate_w: np.ndarray, w1: np.ndarray,
           w2: np.ndarray) -> np.ndarray:
    from concourse.bass_utils import run_bass_kernel_spmd
    import ml_dtypes

    x = np.asarray(x, dtype=np.float32)
    gate_w = np.asarray(gate_w, dtype=np.float32)
    w1 = np.asarray(w1, dtype=np.float32)
    w2 = np.asarray(w2, dtype=np.float32)

    ti, tw = _route(x, gate_w)

    x2d = x.reshape(T, D_MODEL)
    tokens, weights = [], []
    for e in range(N_EXPERTS):
        rows, ks = np.nonzero(ti == e)
        tokens.append(rows)
        weights.append(tw[rows, ks])
    counts = [len(t) for t in tokens]
    C = _round_up(max(max(counts), 1), 128)

    key = (C, COMPUTE_DTYPE)
    if key not in _PROGRAM_CACHE:
        _PROGRAM_CACHE[key] = _build_program(C, COMPUTE_DTYPE)
    nc = _PROGRAM_CACHE[key]

    np_dt = ml_dtypes.bfloat16 if COMPUTE_DTYPE == "bf16" else np.float32

    in_maps = []
    for e in range(N_EXPERTS):
        n = counts[e]
        xt = np.zeros((D_MODEL, C), dtype=np_dt)
        if n:
            xt[:, :n] = x2d[tokens[e]].astype(np_dt).T
        cw = np.zeros((C,), dtype=np.float32)
        cw[:n] = weights[e]
        in_maps.append({
            "xt": xt,
            "w1": w1[e].astype(np_dt),
            "w2": w2[e].astype(np_dt),
            "cw": cw,
        })

    res = run_bass_kernel_spmd(nc, in_maps, core_ids=list(range(N_CORES)))

    out2d = np.zeros((T, D_MODEL), dtype=np.float32)
    for e in range(N_EXPERTS):
        n = counts[e]
        if n:
            out2d[tokens[e]] += res.results[e]["y"][:n]

    LAST_BUILD["nc"] = nc
    LAST_BUILD["C"] = C
    return out2d.reshape(B, S, D_MODEL)


# revision 16
# speedup vs baseline: 1.1044x; 1.1044x over previous
"""MoE (top-2 of 8 experts) Trainium2 kernel.

Strategy: expert-parallel across the 8 NeuronCores. The (cheap) router runs
on host CPU; the host gathers each expert's routed tokens (already
transposed to [D, C] so the device needs no transposes), each core runs the
dense expert MLP  y = (silu(x @ w1_e) @ w2_e) * combine_weight  for its
expert's tokens only (~T*K/E tokens instead of all T — a 4x compute saving
over the dense formulation), and the host scatter-adds the per-expert
contributions back into the full [B,S,D] output.

Self-contained: only environment packages (numpy/jax/concourse) are used.
"""

import os
import sys

import numpy as np

# concourse ships on sys.path via the container's sitecustomize
# (/root/.axon_site/_ro/trn_rl_repo); /opt copy is a fallback only.
if "/opt/trn_rl_repo" not in sys.path:
    sys.path.append("/opt/trn_rl_repo")

B, S, D_MODEL, D_FF, N_EXPERTS, TOP_K = 2, 2048, 1024, 2048, 8, 2
T = B * S
N_CORES = 8

# compute dtype for the expert MLP matmuls: "bf16", "f32", or "f32r"
COMPUTE_DTYPE = os.environ.get("BASS_MOE_DTYPE", "bf16")

_PROGRAM_CACHE: dict = {}
LAST_BUILD = {}


def _round_up(v: int, m: int) -> int:
    return ((v + m - 1) // m) * m


def _blocks(C: int):
    """Token blocks of <=512 (PSUM-bank limit on matmul free dim)."""
    out = []
    b0 = 0
    while b0 < C:
        bs = min(512, C - b0)
        out.append((b0, bs))
        b0 += bs
    return out


def _build_program(C: int, cdtype: str, repeat: int = 1,
                   timing_only: bool = False):
    """Build + compile the per-core expert-MLP program for capacity C.

    repeat>1 wraps the compute in a device-side loop re-running the same
    work; used only for wall-clock HW timing (results unchanged).
    timing_only=True swaps the big I/O tensors for Internal DRAM scratch
    (garbage data) so per-call host<->device transfer is negligible.
    """
    import contextlib
    import concourse.tile as tile
    from concourse import bacc, mybir

    if cdtype == "bf16":
        mdt = mybir.dt.bfloat16
    elif cdtype == "f32":
        mdt = mybir.dt.float32
    elif cdtype == "f32r":
        mdt = mybir.dt.float32r
    else:
        raise ValueError(cdtype)

    f32 = mybir.dt.float32
    KD = D_MODEL // 128   # 8  k-chunks for matmul 1
    KF = D_FF // 128      # 16 k-chunks for matmul 2

    nc = bacc.Bacc("TRN2", target_bir_lowering=False, debug=False,
                   num_devices=N_CORES)
    ik = "Internal" if timing_only else "ExternalInput"
    ok = "Internal" if timing_only else "ExternalOutput"
    xt_d = nc.dram_tensor("xt", [D_MODEL, C], mdt, kind=ik).ap()
    w1_d = nc.dram_tensor("w1", [D_MODEL, D_FF], mdt, kind=ik).ap()
    w2_d = nc.dram_tensor("w2", [D_FF, D_MODEL], mdt, kind=ik).ap()
    cw_d = nc.dram_tensor("cw", [C], f32, kind=ik).ap()
    y_d = nc.dram_tensor("y", [C, D_MODEL], f32, kind=ok).ap()
    if timing_only:
        tin = nc.dram_tensor("tin", [128, 1], f32, kind="ExternalInput").ap()
        tout = nc.dram_tensor("tout", [128, 1], f32, kind="ExternalOutput").ap()

    silu = mybir.ActivationFunctionType.Silu

    with tile.TileContext(nc) as tc:
        with (
            tc.tile_pool(name="wpool", bufs=1) as wpool,
            tc.tile_pool(name="xpool", bufs=2) as xpool,
            tc.tile_pool(name="hpool", bufs=2) as hpool,
            tc.tile_pool(name="cwpool", bufs=3) as cwpool,
            tc.tile_pool(name="ypool", bufs=3) as ypool,
            tc.tile_pool(name="pspool", bufs=1, space="PSUM") as pspool,
        ):
            # resident weights; DMAs split into column chunks and emitted
            # around the first token DMAs so the PE can start ~3us in:
            # w1 cols 0:512 -> (body emits first xt group) -> rest of w1
            # by 512-col chunks -> w2.
            w1_sb = [wpool.tile([128, D_FF], mdt, tag=f"w1_{kk}",
                                name=f"w1t{kk}") for kk in range(KD)]
            w2_sb = [wpool.tile([128, D_MODEL], mdt, tag=f"w2_{kk}",
                                name=f"w2t{kk}") for kk in range(KF)]

            def emit_weight_pre():
                for kk in range(KD):
                    nc.sync.dma_start(w1_sb[kk][:, 0:512],
                                      w1_d[kk * 128:(kk + 1) * 128, 0:512])

            def emit_weight_rest():
                for c0 in range(512, D_FF, 512):
                    for kk in range(KD):
                        nc.sync.dma_start(
                            w1_sb[kk][:, c0:c0 + 512],
                            w1_d[kk * 128:(kk + 1) * 128, c0:c0 + 512])
                for kk in range(KF):
                    nc.sync.dma_start(w2_sb[kk][:],
                                      w2_d[kk * 128:(kk + 1) * 128, :])

            if timing_only:
                tsb = cwpool.tile([128, 1], f32, tag="tsb")
                nc.sync.dma_start(tsb[:], tin[:])
            if repeat > 1:
                # weights resident across iterations; load them up front
                emit_weight_pre()
                emit_weight_rest()
                weight_hook = None
            else:
                weight_hook = (emit_weight_pre, emit_weight_rest)
            rep_ctx = (tc.For_i(0, repeat, 1) if repeat > 1
                       else contextlib.nullcontext())
            with rep_ctx:
                _emit_body(nc, tc, C, mdt, f32, silu, KD, KF,
                           w1_sb, w2_sb, xpool, hpool, cwpool, ypool, pspool,
                           xt_d, cw_d, y_d, weight_hook)
            if timing_only:
                nc.sync.dma_start(tout[:], tsb[:])

    nc.compile()
    return nc


def _emit_body(nc, tc, C, mdt, f32, silu, KD, KF, w1_sb, w2_sb,
               xpool, hpool, cwpool, ypool, pspool, xt_d, cw_d, y_d,
               weight_hook=None):
    """One full pass over the C tokens.

    Token blocks of <=512 are processed in groups of up to GROUP blocks;
    within a group matmul-1 iterates (ff, k) in the outer loops and blocks
    innermost, so each stationary-weight load is reused across the group's
    blocks (LDWEIGHTS count /GROUP on the PE).
    """
    GROUP = 3
    blocks = _blocks(C)
    n_tm_total = C // 128

    if weight_hook is not None:
        weight_hook[0]()  # first w1 column chunk

    # whole combine-weight vector, one DMA: [128, C/128] (token t = col t//128,
    # partition t%128)
    cw_sb = cwpool.tile([128, n_tm_total], f32, tag="cw")
    nc.sync.dma_start(cw_sb[:], cw_d.rearrange("(n p) -> p n", p=128))

    gi = 0
    while gi < len(blocks):
        grp = blocks[gi:gi + GROUP]
        gi += GROUP
        nb = len(grp)

        # stream the group's tokens in (already transposed on host)
        xt_sb = {}
        for bi, (b0, bs) in enumerate(grp):
            for kk in range(KD):
                t = xpool.tile([128, bs], mdt, tag=f"x{kk}_{bi}")
                nc.sync.dma_start(
                    t[:], xt_d[kk * 128:(kk + 1) * 128, b0:b0 + bs])
                xt_sb[kk, bi] = t

        if weight_hook is not None and gi == len(blocks[:GROUP]):
            weight_hook[1]()  # rest of the weights, behind the first xt group

        # h[ff, tok] = silu(w1.T-slice @ x) — stored transposed so it can be
        # the stationary operand of matmul 2. Blocks innermost: one w1
        # stationary load serves nb matmuls.
        h_sb = {}
        for fm in range(KF):
            pss = []
            for bi in range(nb):
                ps = pspool.tile([128, grp[bi][1]], f32, tag="ps",
                                 bufs=8, name=f"ps_h{bi}")
                pss.append(ps)
            for kk in range(KD):
                for bi in range(nb):
                    nc.tensor.matmul(
                        pss[bi][:],
                        lhsT=w1_sb[kk][:, fm * 128:(fm + 1) * 128],
                        rhs=xt_sb[kk, bi][:],
                        start=(kk == 0), stop=(kk == KD - 1))
            for bi in range(nb):
                h = hpool.tile([128, grp[bi][1]], mdt, tag=f"h{fm}_{bi}")
                nc.scalar.activation(h[:], pss[bi][:], silu)
                h_sb[fm, bi] = h

        # y[tok, d] = (h.T @ w2) * cw[tok]
        # fk outer / dn inner: each h stationary load serves both dn tiles
        ND = D_MODEL // 512
        for bi, (b0, bs) in enumerate(grp):
            for tm in range(bs // 128):
                t0 = b0 + tm * 128
                ys = ypool.tile([128, D_MODEL], f32, tag="y")
                pys = []
                for dn in range(ND):
                    py = pspool.tile([128, 512], f32, tag="ps", bufs=8,
                                     name=f"ps_y{dn}")
                    pys.append(py)
                for fk in range(KF):
                    for dn in range(ND):
                        nc.tensor.matmul(
                            pys[dn][:],
                            lhsT=h_sb[fk, bi][:, tm * 128:(tm + 1) * 128],
                            rhs=w2_sb[fk][:, dn * 512:(dn + 1) * 512],
                            start=(fk == 0), stop=(fk == KF - 1))
                for dn in range(ND):
                    # out = psum * combine_weight (per-partition scalar)
                    nc.scalar.mul(ys[:, dn * 512:(dn + 1) * 512], pys[dn][:],
                                  cw_sb[:, t0 // 128:t0 // 128 + 1])
                nc.sync.dma_start(y_d[t0:t0 + 128, :], ys[:])


def _route(x: np.ndarray, gate_w: np.ndarray):
    """Router on host CPU with the reference's exact jax ops/dtypes."""
    try:
        import jax
        import jax.numpy as jnp
        with jax.default_device(jax.devices("cpu")[0]):
            logits = jnp.einsum('bsd,de->bse', jnp.asarray(x),
                                jnp.asarray(gate_w))
            top_logits, top_idx = jax.lax.top_k(logits, TOP_K)
            top_w = jax.nn.softmax(top_logits, axis=-1)
            ti = np.asarray(top_idx).reshape(T, TOP_K)
            tw = np.asarray(top_w).reshape(T, TOP_K).astype(np.float32)
    except Exception:
        # numpy fallback (same selection semantics as jax.lax.top_k)
        logits = (x.reshape(T, D_MODEL) @ gate_w).astype(np.float32)
        i0 = np.argmax(logits, axis=1)
        masked = logits.copy()
        masked[np.arange(T), i0] = -np.inf
        i1 = np.argmax(masked, axis=1)
        v0 = logits[np.arange(T), i0]
        v1 = logits[np.arange(T), i1]
        e1 = np.exp(v1 - v0)
        w0 = 1.0 / (1.0 + e1)
        ti = np.stack([i0, i1], 1)
        tw = np.stack([w0, 1.0 - w0], 1).astype(np.float32)
    return ti, tw


def kernel(x: np.ndarray, gate_w: np.ndarray, w1: np.ndarray,
           w2: np.ndarray) -> np.ndarray:
    from concourse.bass_utils import run_bass_kernel_spmd
    import ml_dtypes

    x = np.asarray(x, dtype=np.float32)
    gate_w = np.asarray(gate_w, dtype=np.float32)
    w1 = np.asarray(w1, dtype=np.float32)
    w2 = np.asarray(w2, dtype=np.float32)

    ti, tw = _route(x, gate_w)

    x2d = x.reshape(T, D_MODEL)
    tokens, weights = [], []
    for e in range(N_EXPERTS):
        rows, ks = np.nonzero(ti == e)
        tokens.append(rows)
        weights.append(tw[rows, ks])
    counts = [len(t) for t in tokens]
    C = _round_up(max(max(counts), 1), 128)

    key = (C, COMPUTE_DTYPE)
    if key not in _PROGRAM_CACHE:
        _PROGRAM_CACHE[key] = _build_program(C, COMPUTE_DTYPE)
    nc = _PROGRAM_CACHE[key]

    np_dt = ml_dtypes.bfloat16 if COMPUTE_DTYPE == "bf16" else np.float32

    in_maps = []
    for e in range(N_EXPERTS):
        n = counts[e]
        xt = np.zeros((D_MODEL, C), dtype=np_dt)
        if n:
            xt[:, :n] = x2d[tokens[e]].astype(np_dt).T
        cw = np.zeros((C,), dtype=np.float32)
        cw[:n] = weights[e]
        in_maps.append({
            "xt": xt,
            "w1": w1[e].astype(np_dt),
            "w2": w2[e].astype(np_dt),
            "cw": cw,
        })

    res = run_bass_kernel_spmd(nc, in_maps, core_ids=list(range(N_CORES)))

    out2d = np.zeros((T, D_MODEL), dtype=np.float32)
    for e in range(N_EXPERTS):
        n = counts[e]
        if n:
            out2d[tokens[e]] += res.results[e]["y"][:n]

    LAST_BUILD["nc"] = nc
    LAST_BUILD["C"] = C
    return out2d.reshape(B, S, D_MODEL)
